# revision 3
# baseline (speedup 1.0000x reference)
"""Trainium2 Bass kernel v3 for nn_AttentionPermMatrix (Sinkhorn sampling).

v3 over v2 (83 us):
  - PE mean work is the wall (LDW+MM serialize at ~95 ns/chunk): offload the
    last pair's q/k means to DVE via a b-innermost host layout + segmented
    tensor_reduce (chunked so sinkhorn reciprocals can interleave).
  - ACT table thrash (9 loads x 1.3 us): pairs processed in supergroups of
    two with Ln/Ln/Exp/Exp ordering; psum copies stay on ACT (Copy is in
    every table set).
  - Reciprocals batched [128, 2] across the 2-pair group; block-diag zero
    memsets only on first use of each pool buffer.
"""
import math
from contextlib import ExitStack

import numpy as np
import ml_dtypes

import concourse.bass as bass
import concourse.tile as tile
from concourse import bacc, mybir
from concourse.bass_utils import run_bass_kernel_spmd
from concourse.masks import make_identity

F32 = mybir.dt.float32
F16 = mybir.dt.float16
FP8 = mybir.dt.float8e4
AF = mybir.ActivationFunctionType
AX = mybir.AxisListType
OP = mybir.AluOpType

F8NP = ml_dtypes.float8_e4m3

BLOCK, E, BLOCKS = 128, 64, 64
FB = E * BLOCKS
PFB = 2 * FB                 # 8192 per pair
TEMP = 0.7
N_ITERS = 8
EPS = 1e-6
C_LNS = -0.5 * math.log(float(BLOCK))
N_CORES = 8
INV_B = 1.0 / BLOCK


def dve_pairs_for(NP):
    """Pairs whose means run on DVE (b-innermost layout) instead of PE."""
    return {NP - 1} if NP >= 2 else set()


def emit(tc, q, k, g, out, S):
    nc = tc.nc
    NP = S // 2
    DVE_PAIRS = dve_pairs_for(NP)
    groups = [list(range(i, min(i + 2, NP))) for i in range(0, NP, 2)]
    with ExitStack() as ctx:
        ctx.enter_context(nc.allow_low_precision(
            reason="fp8 inputs + f32r/f16 matmuls; validated vs 2e-2 gate"))
        consts = ctx.enter_context(tc.tile_pool(name="consts", bufs=1))
        glob = ctx.enter_context(tc.tile_pool(name="glob", bufs=1))
        qk = ctx.enter_context(tc.tile_pool(name="qk", bufs=3))
        work = ctx.enter_context(tc.tile_pool(name="work", bufs=2))
        qpool = ctx.enter_context(tc.tile_pool(name="qpool", bufs=2))
        uv = ctx.enter_context(tc.tile_pool(name="uv", bufs=3))
        osp = ctx.enter_context(tc.tile_pool(name="osp", bufs=2))
        ps_pm = ctx.enter_context(tc.tile_pool(name="ps_pm", bufs=2, space="PSUM"))
        ps_r = ctx.enter_context(tc.tile_pool(name="ps_r", bufs=2, space="PSUM"))
        ps_t = ctx.enter_context(tc.tile_pool(name="ps_t", bufs=2, space="PSUM"))
        ps_mv = ctx.enter_context(tc.tile_pool(name="ps_mv", bufs=2, space="PSUM"))

        ones8 = consts.tile([BLOCK, 1], FP8)
        nc.vector.memset(ones8, INV_B)
        ones16 = consts.tile([BLOCK, 1], F16)
        nc.vector.memset(ones16, 1.0)
        ident32 = consts.tile([128, 128], F32)
        make_identity(nc, ident32)
        eps_col = consts.tile([BLOCK, 1], F32)
        nc.vector.memset(eps_col, EPS)

        # gumbel prologue: hb = C - ln(-ln(u+eps)+eps)
        gt = glob.tile([BLOCK, NP, BLOCKS], F32)
        nc.sync.dma_start(out=gt, in_=g.ap())
        ga = glob.tile([BLOCK, NP, BLOCKS], F32)
        nc.scalar.activation(ga, gt, AF.Ln, bias=eps_col[:], scale=1.0)
        gb = glob.tile([BLOCK, NP, BLOCKS], F32)
        nc.scalar.activation(gb, ga, AF.Ln, bias=eps_col[:], scale=-1.0)
        hb = glob.tile([BLOCK, NP, BLOCKS], F32)
        nc.vector.tensor_scalar(out=hb, in0=gb, scalar1=-1.0, scalar2=C_LNS,
                                op0=OP.mult, op1=OP.add)

        memset_count = {}

        def memset_if_fresh(t, tag, bufs=2):
            n = memset_count.get(tag, 0)
            memset_count[tag] = n + 1
            if n < bufs:
                nc.vector.memset(t, 0.0)

        def phase_a(pr):
            """loads + means + R + Ln placement into rln tile; returns tiles."""
            if pr in DVE_PAIRS:
                # b-innermost layout [128=(a,e), 64 i, 128 b]: DVE reduce
                qt = qk.tile([BLOCK, BLOCKS, BLOCK], FP8, tag="qtr")
                nc.sync.dma_start(out=qt, in_=q.ap()[:, pr, :])
                kt = qk.tile([BLOCK, BLOCKS, BLOCK], FP8, tag="ktr")
                nc.scalar.dma_start(out=kt, in_=k.ap()[:, pr, :])
                pmr_q = work.tile([BLOCK, BLOCKS], F32, tag="pmrq")
                pmr_k = work.tile([BLOCK, BLOCKS], F32, tag="pmrk")
                # chunked so recips of other pairs can interleave on DVE
                for c0 in range(0, BLOCKS, 8):
                    nc.vector.tensor_reduce(
                        out=pmr_q[:, c0:c0 + 8], in_=qt[:, c0:c0 + 8, :],
                        axis=AX.X, op=OP.add)
                for c0 in range(0, BLOCKS, 8):
                    nc.vector.tensor_reduce(
                        out=pmr_k[:, c0:c0 + 8], in_=kt[:, c0:c0 + 8, :],
                        axis=AX.X, op=OP.add)
                pmq_bd = work.tile([BLOCK, BLOCK], F16, tag="pmq")
                memset_if_fresh(pmq_bd, "pmq")
                for a in range(2):
                    sl = slice(64 * a, 64 * a + 64)
                    nc.vector.tensor_scalar(out=pmq_bd[sl, sl], in0=pmr_q[sl, :],
                                            scalar1=INV_B, scalar2=None,
                                            op0=OP.mult)
                pmk_sb = work.tile([BLOCK, BLOCKS], F16, tag="pmk")
                nc.vector.tensor_scalar(out=pmk_sb[:], in0=pmr_k[:],
                                        scalar1=INV_B, scalar2=None,
                                        op0=OP.mult)
            else:
                qt = qk.tile([BLOCK, PFB], FP8, tag="qt")
                nc.sync.dma_start(out=qt[:], in_=q.ap()[:, pr, :])
                kt = qk.tile([BLOCK, PFB], FP8, tag="kt")
                nc.scalar.dma_start(out=kt[:], in_=k.ap()[:, pr, :])
                pm_q = ps_pm.tile([BLOCK, BLOCKS], F32, tag="pm")
                for c in range(BLOCKS):
                    nc.tensor.matmul(pm_q[:, c:c + 1],
                                     lhsT=qt[:, 128 * c:128 * (c + 1)],
                                     rhs=ones8[:], start=True, stop=True)
                pm_k = ps_pm.tile([BLOCK, BLOCKS], F32, tag="pm")
                for c in range(BLOCKS):
                    nc.tensor.matmul(pm_k[:, c:c + 1],
                                     lhsT=kt[:, 128 * c:128 * (c + 1)],
                                     rhs=ones8[:], start=True, stop=True)
                pmq_bd = work.tile([BLOCK, BLOCK], F16, tag="pmq")
                memset_if_fresh(pmq_bd, "pmq")
                for a in range(2):
                    sl = slice(64 * a, 64 * a + 64)
                    nc.vector.tensor_copy(pmq_bd[sl, sl], pm_q[sl, :])
                pmk_sb = work.tile([BLOCK, BLOCKS], F16, tag="pmk")
                nc.scalar.copy(pmk_sb[:], pm_k[:])

            rp = ps_r.tile([BLOCK, BLOCKS], F32, tag="r")
            nc.tensor.matmul(rp[:], lhsT=pmq_bd[:], rhs=pmk_sb[:],
                             start=True, stop=True)
            return rp

        for grp in groups:
            rps = [phase_a(pr) for pr in grp]
            # Ln/Ln then Exp/Exp: minimizes ACT table switches
            rlns = []
            for j, pr in enumerate(grp):
                rln = work.tile([BLOCK, BLOCKS], F32, tag=f"rln{j}")
                nc.scalar.activation(rln[:], rps[j][:], AF.Ln)
                rlns.append(rln)
            p0s = []
            for j, pr in enumerate(grp):
                tsum = work.tile([BLOCK, BLOCKS], F32, tag=f"tsum{j}")
                nc.vector.tensor_add(tsum[:], rlns[j][:], hb[:, pr, :])
                p0 = work.tile([BLOCK, BLOCKS], F32, tag=f"p0{j}")
                nc.scalar.activation(p0[:], tsum[:], AF.Exp, scale=1.0 / TEMP)
                p0s.append(p0)

            gn = len(grp)
            # fold u1 + block-diag builds per pair
            qp16s, qp32s, qt16s, qt32s = [], [], [], []
            for j, pr in enumerate(grp):
                rs = uv.tile([BLOCK, 1], F32, tag=f"rs{j}")
                nc.vector.reduce_sum(rs[:], p0s[j][:], axis=AX.X)
                u1 = uv.tile([BLOCK, 1], F32, tag=f"u1{j}")
                nc.vector.reciprocal(u1[:], rs[:])
                qp16 = qpool.tile([128, 128], F16, tag=f"qp16_{j}")
                memset_if_fresh(qp16, f"qp16_{j}")
                qp32 = qpool.tile([128, 128], F32, tag=f"qp32_{j}")
                memset_if_fresh(qp32, f"qp32_{j}")
                for a in range(2):
                    sl = slice(64 * a, 64 * a + 64)
                    nc.vector.tensor_scalar(out=qp32[sl, sl], in0=p0s[j][sl, :],
                                            scalar1=u1[sl, :], scalar2=None,
                                            op0=OP.mult)
                    nc.vector.tensor_scalar(out=qp16[sl, sl], in0=p0s[j][sl, :],
                                            scalar1=u1[sl, :], scalar2=None,
                                            op0=OP.mult)
                pt32 = ps_t.tile([128, 128], F32, tag="t")
                nc.tensor.transpose(pt32[:], qp32[:], ident32[:])
                qt16 = qpool.tile([128, 128], F16, tag=f"qt16_{j}")
                nc.vector.tensor_copy(qt16[:], pt32[:])
                qt32 = qpool.tile([128, 128], F32, tag=f"qt32_{j}")
                nc.scalar.copy(qt32[:], pt32[:])
                qp16s.append(qp16); qp32s.append(qp32)
                qt16s.append(qt16); qt32s.append(qt32)

            # Sinkhorn lockstep across the group, recips batched [128, gn]
            u16 = u32 = v16 = v32 = None
            for it in range(N_ITERS):
                f32v = it >= N_ITERS - 2
                pv = ps_mv.tile([BLOCK, gn], F32, tag="mv")
                for j in range(gn):
                    if it == 0:
                        nc.tensor.matmul(pv[:, j:j + 1], lhsT=qp16s[j][:],
                                         rhs=ones16[:], start=True, stop=True)
                    elif f32v:
                        nc.tensor.matmul(pv[:, j:j + 1], lhsT=qp32s[j][:],
                                         rhs=u32[:, j:j + 1], start=True, stop=True)
                    else:
                        nc.tensor.matmul(pv[:, j:j + 1], lhsT=qp16s[j][:],
                                         rhs=u16[:, j:j + 1], start=True, stop=True)
                if f32v:
                    v32 = uv.tile([BLOCK, gn], F32, tag="v32")
                    nc.vector.reciprocal(v32[:], pv[:])
                else:
                    v16 = uv.tile([BLOCK, gn], F16, tag="v16")
                    nc.vector.reciprocal(v16[:], pv[:])
                if it < N_ITERS - 1:
                    pu = ps_mv.tile([BLOCK, gn], F32, tag="mv")
                    for j in range(gn):
                        if it == N_ITERS - 2:
                            nc.tensor.matmul(pu[:, j:j + 1], lhsT=qt32s[j][:],
                                             rhs=v32[:, j:j + 1], start=True, stop=True)
                        else:
                            nc.tensor.matmul(pu[:, j:j + 1], lhsT=qt16s[j][:],
                                             rhs=v16[:, j:j + 1], start=True, stop=True)
                    if it >= N_ITERS - 3:
                        u32 = uv.tile([BLOCK, gn], F32, tag="u32")
                        nc.vector.reciprocal(u32[:], pu[:])
                    else:
                        u16 = uv.tile([BLOCK, gn], F16, tag="u16")
                        nc.vector.reciprocal(u16[:], pu[:])

            # out = diag(u8) Q diag(v8) per pair
            for j, pr in enumerate(grp):
                dv = qpool.tile([128, 128], F32, tag=f"dv{j}")
                nc.vector.tensor_scalar(out=dv[:], in0=ident32[:],
                                        scalar1=v32[:, j:j + 1], scalar2=None,
                                        op0=OP.mult)
                OS = ps_t.tile([128, 128], F32, tag="t")
                nc.tensor.matmul(OS[:], lhsT=qt32s[j][:], rhs=dv[:],
                                 start=True, stop=True)
                os_c = osp.tile([BLOCK, BLOCKS], F32, tag="os")
                for a in range(2):
                    sl = slice(64 * a, 64 * a + 64)
                    nc.vector.tensor_scalar(out=os_c[sl, :], in0=OS[sl, sl],
                                            scalar1=u32[sl, j:j + 1],
                                            scalar2=None, op0=OP.mult)
                nc.gpsimd.dma_start(out=out.ap()[2 * pr:2 * pr + 2], in_=os_c[:])


def build_nc(S=8):
    nc = bacc.Bacc("TRN2", target_bir_lowering=False, debug=False)
    q = nc.dram_tensor("q", [BLOCK, S // 2, PFB], FP8, kind="ExternalInput")
    k = nc.dram_tensor("k", [BLOCK, S // 2, PFB], FP8, kind="ExternalInput")
    g = nc.dram_tensor("g", [BLOCK, S // 2, BLOCKS], F32, kind="ExternalInput")
    out = nc.dram_tensor("out", [S, BLOCKS, BLOCKS], F32, kind="ExternalOutput")
    with tile.TileContext(nc) as tc:
        emit(tc, q, k, g, out, S)
    nc.compile()
    return nc


_NC_CACHE = {}
LAST_RESULTS = None


def prep_inputs(b_q, b_k, gumbel_u, n_cores=N_CORES):
    B = np.asarray(b_q).shape[0]
    S = B // n_cores
    NP = S // 2
    DVE_PAIRS = dve_pairs_for(NP)
    q8 = np.asarray(b_q, dtype=np.float32).astype(F8NP)
    k8 = np.asarray(b_k, dtype=np.float32).astype(F8NP)

    def layout(arr8):
        base = arr8.reshape(n_cores, NP, 2, BLOCK, E, BLOCKS)  # c,pr,a,b,e,i
        slabs = []
        for pr in range(NP):
            s = base[:, pr]
            if pr in DVE_PAIRS:
                # [c, a, e, i, b] -> partition (a,e), free (i, b-innermost)
                t = s.transpose(0, 1, 3, 4, 2).reshape(n_cores, 1, BLOCK, PFB)
            else:
                # [c, b, i, a, e] -> partition b, free i*128 + a*64 + e
                t = s.transpose(0, 2, 4, 1, 3).reshape(n_cores, 1, BLOCK, PFB)
            slabs.append(t)
        return np.ascontiguousarray(
            np.concatenate(slabs, axis=1).transpose(0, 2, 1, 3))

    qh = layout(q8)
    kh = layout(k8)
    g = np.asarray(gumbel_u, dtype=np.float32)
    g2 = np.ascontiguousarray(
        g.reshape(n_cores, NP, 2, BLOCKS, BLOCKS).transpose(0, 2, 3, 1, 4)
    ).reshape(n_cores, BLOCK, NP, BLOCKS)
    return qh, kh, g2, S


def kernel(b_q, b_k, gumbel_u, _trace=False):
    global LAST_RESULTS
    qh, kh, g2, S = prep_inputs(b_q, b_k, gumbel_u)
    if S not in _NC_CACHE:
        _NC_CACHE[S] = build_nc(S)
    nc = _NC_CACHE[S]
    in_maps = [{"q": qh[c], "k": kh[c], "g": g2[c]} for c in range(N_CORES)]
    res = run_bass_kernel_spmd(nc, in_maps, core_ids=list(range(N_CORES)),
                               trace=_trace)
    LAST_RESULTS = res
    return np.concatenate([r["out"] for r in res.results], axis=0)
